# revision 9
# baseline (speedup 1.0000x reference)
"""Trainium2 Bass kernel for nn_ChallengingGeometricLoss.

Computes loss = 0.1 * mean(exp(-0.1 * cdist(x, x)))  for x = embeddings
reshaped to [N=8192, d=512], plus total = 0.5 * loss.

Key idea: approximate the scalar map  t -> exp(-0.1*sqrt(t))  (t = squared
distance) by  exp(gamma - (alpha*t + beta)^2)  — a least-squares fit of
-0.1*sqrt(t) by a concave parabola over the data's t-range (fit on host
from ~400K sampled pairs; induced relative bias of the mean ~5e-5).  Then
the whole elementwise tail is ONE activation pass:

    g = Derivative_Erf(alpha * psum + bias_i) = (2/sqrt(pi)) * exp(-h^2)

with h = alpha*t + beta assembled by the ACT free affine: psum holds
(a_j - abar) - 2*p_ij from the PE (column norm encoded as an extra fp8
k-row: stationary row 511 = 64.0, moving row 511 = (a_j-abar)/64, x dim
511 dropped — zero-mean noise), and bias_i = alpha*(a_i + abar) + beta is
the per-partition bias.  accum_out gives the band sums for free.

Coverage (8 cores, SPMD): 64 row-blocks of 128; row-block r covers column
blocks (r+delta)%64 for delta in 0..31 (4096 cols; psum double-buffered).
Band 0 is split into 4 [128,1024] psum groups so ACT starts early; bands
1-7 use 2 [128,2048] groups.  The true diagonal of the delta=0 block is
masked with a -60000 PE matmul (h ~ -15 -> g = 0 exactly); delta=0 block
sums (E0) are re-reduced on the idle DVE.  The 32 delta=32 pairs are a
separate uniform X-group (4 [128,128] blocks/core) with BOTH norms encoded
in k-rows 510/511 so the bias is constant.  Host combine:
    sum_full = K*(2*(T+X) - E0) + N,  K = exp(gamma)*sqrt(pi)/2
    loss = 0.1 * sum_full / N^2
"""

import ml_dtypes
import numpy as np

import concourse.bass as bass  # noqa: F401
import concourse.mybir as mybir
import concourse.tile as tile
from concourse import bacc
from concourse.bass_utils import run_bass_kernel_spmd

# Problem constants (hardcoded per contract).
N = 8192
D = 512
NCORES = 8
P = 128
KC = D // P            # 4 k-chunks of 128
NB = 8                 # 128-row blocks per core
BAND = 4096            # cyclic band (delta 0..31)
HALF = 2048
WIN = 4992             # per-core moving window: 39 blocks
NX = 4                 # delta-32 blocks per core
BIGVAL = 60000.0       # diagonal mask magnitude (exact in fp16)
ENC = 64.0             # norm-encode scale (exact in fp8)

dt = mybir.dt
AF = mybir.ActivationFunctionType


def build_program():
    """Build the per-core Bass/Tile program (identical across cores)."""
    nc = bacc.Bacc("TRN2", num_devices=NCORES, debug=False)

    f8 = dt.float8e4
    # DRAM layout is per-k [KC, P, cols] so each DMA piece moves 128
    # partition-chunks of >=2KB contiguous bytes (fragmented DMAs measure
    # ~20GB/s; contiguous ~250GB/s).  Issues cost ~650ns of engine time
    # each, so they are spread over the sync/gpsimd/tensor/scalar queues
    # (tensor and scalar are otherwise idle during the DMA window).
    xst_d = nc.dram_tensor("xst", [KC, P, NB * P], f8, kind="ExternalInput")
    xmov_d = nc.dram_tensor("xmov", [KC, P, WIN], f8, kind="ExternalInput")
    xx_d = nc.dram_tensor("xx", [KC, P, 2 * NX * P], f8, kind="ExternalInput")
    consts_d = nc.dram_tensor("consts", [P, 16], dt.float32, kind="ExternalInput")
    masks_d = nc.dram_tensor("masks", [P, 2 * P], dt.float16, kind="ExternalInput")
    out_d = nc.dram_tensor("outacc", [P, 27], dt.float32, kind="ExternalOutput")

    with tile.TileContext(nc) as tc:
        with (
            tc.tile_pool(name="big", bufs=1) as bigp,
            tc.tile_pool(name="obuf", bufs=3) as obufp,
            tc.tile_pool(name="small", bufs=1) as smallp,
            tc.tile_pool(name="psum", bufs=2, space="PSUM") as psump,
        ):
            xst = bigp.tile([P, KC, NB * P], f8, tag="xst")
            xmov = bigp.tile([P, KC, WIN], f8, tag="xmov")
            xx = bigp.tile([P, KC, 2 * NX * P], f8, tag="xx")
            consts = smallp.tile([P, 16], dt.float32, tag="consts")
            masks = smallp.tile([P, 2 * P], dt.float16, tag="masks")
            acc = smallp.tile([P, 27], dt.float32, tag="acc")

            # ACT table preload: tiny Derivative_Erf on memset tiles (no DMA
            # deps) so the ~2.7us table load runs during the input DMAs.
            wact = smallp.tile([P, 8], dt.float32, tag="wact")
            wbias = smallp.tile([P, 1], dt.float32, tag="wbias")
            nc.vector.memset(wact[:, :], 1.0)
            nc.vector.memset(wbias[:, :], 0.0)
            nc.scalar.activation(wact[:, :], wact[:, :], AF.Derivative_Erf,
                                 bias=wbias[:, :], scale=1.0)

            # Short PE warmup (HAM clock ramp) while the first DMAs land.
            wident = smallp.tile([P, P], dt.float16, tag="wident")
            wmov = smallp.tile([P, 512], dt.float16, tag="wmov")
            nc.vector.memset(wident[:, :], 1.0)
            nc.vector.memset(wmov[:, :], 1.0)
            warm = psump.tile([P, 512], dt.float32, tag="ps")
            for _ in range(3):
                nc.tensor.matmul(warm[:, :], wident[:, :], wmov[:, :],
                                 start=True, stop=True)

            # Input DMAs: group-0 data (xst + xmov[0:2048] all k) first,
            # fanned out over the three DMA-capable queues.  The scalar
            # queue takes only tiny/late pieces so ACT stays free.
            nc.scalar.dma_start(consts[:], consts_d[:])
            nc.scalar.dma_start(masks[:], masks_d[:])
            nc.sync.dma_start(xst[:, 0, :], xst_d[0, :, :])
            nc.gpsimd.dma_start(xst[:, 1, :], xst_d[1, :, :])
            nc.sync.dma_start(xst[:, 2, :], xst_d[2, :, :])
            nc.gpsimd.dma_start(xst[:, 3, :], xst_d[3, :, :])
            nc.sync.dma_start(xmov[:, 0, 0:HALF], xmov_d[0, :, 0:HALF])
            nc.gpsimd.dma_start(xmov[:, 1, 0:HALF], xmov_d[1, :, 0:HALF])
            nc.sync.dma_start(xmov[:, 2, 0:HALF], xmov_d[2, :, 0:HALF])
            nc.gpsimd.dma_start(xmov[:, 3, 0:HALF], xmov_d[3, :, 0:HALF])
            nc.scalar.dma_start(xmov[:, 3, HALF:WIN], xmov_d[3, :, HALF:WIN])
            nc.sync.dma_start(xmov[:, 0, HALF:WIN], xmov_d[0, :, HALF:WIN])
            nc.gpsimd.dma_start(xmov[:, 1, HALF:WIN], xmov_d[1, :, HALF:WIN])
            nc.sync.dma_start(xmov[:, 2, HALF:WIN], xmov_d[2, :, HALF:WIN])
            for k in range(KC):
                nc.gpsimd.dma_start(xx[:, k, :], xx_d[k, :, :])

            ident = masks[:, 0:P]
            negbig = masks[:, P:2 * P]

            def emit_group(u, g0, glen, acc_col):
                """One psum group: band u, band-cols [g0, g0+glen)."""
                ps = psump.tile([P, glen], dt.float32, tag="ps")
                obuf = obufp.tile([P, glen], dt.float32, tag="ob")
                for lo in range(0, glen, 512):
                    col0 = P * u + g0 + lo
                    nc.tensor.matmul(
                        ps[:, lo:lo + 512],
                        xst[:, 0:2, P * u:P * u + P],
                        xmov[:, 0:2, col0:col0 + 512],
                        start=True, stop=False,
                        perf_mode=mybir.MatmulPerfMode.DoubleRow,
                    )
                    if g0 == 0 and lo == 0:
                        # Mask the true diagonal: psum += I.T@(-BIG*I).
                        nc.tensor.matmul(
                            ps[:, 0:P], ident, negbig,
                            start=False, stop=False,
                        )
                    nc.tensor.matmul(
                        ps[:, lo:lo + 512],
                        xst[:, 2:4, P * u:P * u + P],
                        xmov[:, 2:4, col0:col0 + 512],
                        start=False, stop=True,
                        perf_mode=mybir.MatmulPerfMode.DoubleRow,
                    )
                nc.scalar.activation(
                    obuf[:, :], ps[:, :], AF.Derivative_Erf,
                    bias=consts[:, u:u + 1],
                    scale=consts[:, 8:9],
                    accum_out=acc[:, acc_col:acc_col + 1],
                )
                if g0 == 0:
                    # delta=0 block sums (single-counted correction).
                    nc.vector.tensor_reduce(
                        acc[:, 19 + u:20 + u], obuf[:, 0:P],
                        axis=mybir.AxisListType.X, op=mybir.AluOpType.add,
                    )

            def emit_xgroup():
                """The 4 delta=32 blocks; both norms encoded in-psum."""
                psx = psump.tile([P, NX * P], dt.float32, tag="ps")
                obx = obufp.tile([P, NX * P], dt.float32, tag="ob")
                for k in range(NX):
                    lo = k * P
                    nc.tensor.matmul(
                        psx[:, lo:lo + P],
                        xx[:, 0:2, lo:lo + P], xx[:, 0:2, NX * P + lo:NX * P + lo + P],
                        start=True, stop=False,
                        perf_mode=mybir.MatmulPerfMode.DoubleRow,
                    )
                    nc.tensor.matmul(
                        psx[:, lo:lo + P],
                        xx[:, 2:4, lo:lo + P], xx[:, 2:4, NX * P + lo:NX * P + lo + P],
                        start=False, stop=True,
                        perf_mode=mybir.MatmulPerfMode.DoubleRow,
                    )
                nc.scalar.activation(
                    obx[:, :], psx[:, :], AF.Derivative_Erf,
                    bias=consts[:, 9:10], scale=consts[:, 8:9],
                    accum_out=acc[:, 18:19],
                )

            # Band 0 in four 1024-col groups (ACT pipeline fills early),
            # bands 1-7 in two 2048-col groups; X-group mid-stream (its
            # inputs arrive on the gpsimd queue last).
            for g in range(4):
                emit_group(0, 1024 * g, 1024, g)
            nacc = 4
            for u in range(1, NB):
                for half in range(2):
                    emit_group(u, half * HALF, HALF, nacc)
                    nacc += 1
                if u == 4:
                    emit_xgroup()

            # Ship the [128, 27] accumulator; partition reduction on host.
            nc.sync.dma_start(out_d[:], acc[:])

    nc.finalize()
    return nc


def _fit_parabola(a, x):
    """Weighted LSQ fit of -0.1*sqrt(t) ~ gamma - (alpha*t+beta)^2 over the
    empirical distribution of pairwise squared distances t."""
    rng = np.random.default_rng(12345)
    M = 400_000
    i = rng.integers(0, N, M)
    j = rng.integers(0, N, M)
    keep = i != j
    i, j = i[keep], j[keep]
    xf = x.astype(np.float32)
    t = (a[i] + a[j]
         - 2.0 * np.einsum('ij,ij->i', xf[i], xf[j], optimize=True).astype(np.float64))
    z = 0.1 * np.sqrt(np.maximum(t, 0.0))
    w = np.exp(-z)
    # init: pick gamma0, fit h = sqrt(gamma0+z) affine in t by weighted LSQ
    ga = 6.3
    h0 = np.sqrt(ga + z)
    W = w * w
    A = np.stack([t, np.ones_like(t)], 1)
    AtW = A.T * W
    al, be = np.linalg.solve(AtW @ A, AtW @ h0)

    # Levenberg-Marquardt on r = w*(ga - h^2 + z) (plain GN overshoots).
    def cost(al_, be_, ga_):
        h_ = al_ * t + be_
        r_ = w * (ga_ - h_ * h_ + z)
        return float((r_ * r_).sum())

    lam = 1e-3
    for _ in range(30):
        h = al * t + be
        r = w * (ga - h * h + z)
        J = np.stack([-2 * w * h * t, -2 * w * h, w], 1)
        JTJ = J.T @ J
        g = J.T @ r
        c0 = float((r * r).sum())
        while True:
            Hm = JTJ + lam * np.diag(np.diag(JTJ))
            dlt = np.linalg.solve(Hm, -g)
            cand = (al + dlt[0], be + dlt[1], ga + dlt[2])
            if cost(*cand) <= c0 or lam > 1e12:
                break
            lam *= 10.0
        al, be, ga = cand
        lam = max(lam * 0.3, 1e-12)
    return float(al), float(be), float(ga)


def prepare_inputs(x):
    """Host-side sharding: per-core input dicts for run_bass_kernel_spmd."""
    x = np.ascontiguousarray(np.asarray(x, dtype=np.float32).reshape(N, D))
    a = (x.astype(np.float64) ** 2).sum(axis=1)          # true row norms
    abar = float(a.mean())
    al, be, ga = _fit_parabola(a, x)

    f8 = ml_dtypes.float8_e4m3
    da_enc = ((a - abar) / ENC).astype(f8)               # [N] fp8

    # Moving matrix M [512, N]: rows 0..510 = x dims, row 511 = da_enc.
    MT = np.empty((D, N), dtype=f8)
    MT[0:D - 1] = x.T[0:D - 1].astype(f8)
    MT[D - 1] = da_enc
    # Stationary S [512, N]: rows 0..510 = -2x, row 511 = 64.0.
    ST = np.empty((D, N), dtype=f8)
    ST[0:D - 1] = (-2.0 * x.T[0:D - 1]).astype(f8)
    ST[D - 1] = f8(ENC)
    # X variants: rows 0..509 = dims, plus both norm encodes.
    MXT = np.empty((D, N), dtype=f8)
    MXT[0:D - 2] = MT[0:D - 2]
    MXT[D - 2] = f8(ENC)
    MXT[D - 1] = da_enc
    SXT = np.empty((D, N), dtype=f8)
    SXT[0:D - 2] = ST[0:D - 2]
    SXT[D - 2] = da_enc
    SXT[D - 1] = f8(ENC)

    masks = np.zeros((P, 2 * P), dtype=np.float16)
    masks[:, 0:P] = np.eye(P, dtype=np.float16)
    masks[:, P:2 * P] = (-BIGVAL * np.eye(P)).astype(np.float16)

    in_maps = []
    for c in range(NCORES):
        rows = 1024 * c + np.arange(1024)
        win = (1024 * c + np.arange(WIN)) % N
        rx = 512 * c + np.arange(512)
        cx = rx + 4096
        xst = np.ascontiguousarray(ST[:, rows].reshape(KC, P, NB * P))
        xmov = np.ascontiguousarray(MT[:, win].reshape(KC, P, WIN))
        xx = np.empty((KC, P, 2 * NX * P), dtype=f8)
        xx[:, :, 0:NX * P] = SXT[:, rx].reshape(KC, P, NX * P)
        xx[:, :, NX * P:] = MXT[:, cx].reshape(KC, P, NX * P)
        consts = np.zeros((P, 16), dtype=np.float32)
        consts[:, 0:NB] = (al * (a[rows] + abar) + be).astype(np.float32).reshape(NB, P).T
        consts[:, 8] = al
        consts[:, 9] = al * 2.0 * abar + be
        in_maps.append({
            "xst": xst,
            "xmov": xmov,
            "xx": np.ascontiguousarray(xx),
            "consts": consts,
            "masks": masks,
        })
    return in_maps, (al, be, ga)


def combine_outputs(results, ga):
    """Combine per-core [128, 27] accumulators into the final loss values."""
    K = np.exp(ga) * np.sqrt(np.pi) / 2.0
    S = 0.0
    for r in results:
        o = np.asarray(r["outacc"], dtype=np.float64).sum(axis=0)  # [27]
        TX = o[0:19].sum()       # 18 band-group sums + X (all double-counted)
        E0 = o[19:27].sum()      # delta=0 block sums (single-counted)
        S += 2.0 * TX - E0
    total = K * S + float(N)  # exact diagonal (masked to 0 on device)
    loss = 0.1 * total / (float(N) * float(N))
    return np.float32(loss), np.float32(0.5 * loss)


_CACHE = {}


def _get_program():
    if "nc" not in _CACHE:
        _CACHE["nc"] = build_program()
    return _CACHE["nc"]


def run(embeddings, trace=False):
    """Run the Bass kernel on 8 cores; returns (loss, total, BassKernelResults)."""
    nc = _get_program()
    in_maps, (al, be, ga) = prepare_inputs(embeddings)
    res = run_bass_kernel_spmd(nc, in_maps, core_ids=list(range(NCORES)),
                               trace=trace)
    loss, total = combine_outputs(res.results, ga)
    return loss, total, res


def kernel(embeddings):
    loss, total, _ = run(embeddings, trace=False)
    return loss, total


# revision 12
# speedup vs baseline: 1.0226x; 1.0226x over previous
"""Trainium2 Bass kernel for nn_ChallengingGeometricLoss.

Computes loss = 0.1 * mean(exp(-0.1 * cdist(x, x)))  for x = embeddings
reshaped to [N=8192, d=512], plus total = 0.5 * loss.

Key idea: approximate the scalar map  t -> exp(-0.1*sqrt(t))  (t = squared
distance) by  exp(gamma - (alpha*t + beta)^2)  — a least-squares fit of
-0.1*sqrt(t) by a concave parabola over the data's t-range (fit on host
from ~400K sampled pairs; induced relative bias of the mean ~5e-5).  Then
the whole elementwise tail is ONE activation pass:

    g = Derivative_Erf(alpha * psum + bias_i) = (2/sqrt(pi)) * exp(-h^2)

with h = alpha*t + beta assembled by the ACT free affine: psum holds
(a_j - abar) - 2*p_ij from the PE (column norm encoded as an extra fp8
k-row: stationary row 511 = 64.0, moving row 511 = (a_j-abar)/64, x dim
511 dropped — zero-mean noise), and bias_i = alpha*(a_i + abar) + beta is
the per-partition bias.  accum_out gives the band sums for free.

Coverage (8 cores, SPMD): 64 row-blocks of 128; row-block r covers column
blocks (r+delta)%64 for delta in 0..31 (4096 cols; psum double-buffered).
Band 0 is split into 4 [128,1024] psum groups so ACT starts early; bands
1-7 use 2 [128,2048] groups.  The true diagonal of the delta=0 block is
masked with a -60000 PE matmul (h ~ -15 -> g = 0 exactly); delta=0 block
sums (E0) are re-reduced on the idle DVE.  The 32 delta=32 pairs are a
separate uniform X-group (4 [128,128] blocks/core) with BOTH norms encoded
in k-rows 510/511 so the bias is constant.  Host combine:
    sum_full = K*(2*(T+X) - E0) + N,  K = exp(gamma)*sqrt(pi)/2
    loss = 0.1 * sum_full / N^2
"""

import ml_dtypes
import numpy as np

import concourse.bass as bass  # noqa: F401
import concourse.mybir as mybir
import concourse.tile as tile
from concourse import bacc
from concourse.bass_utils import run_bass_kernel_spmd

# Problem constants (hardcoded per contract).
N = 8192
D = 512
NCORES = 8
P = 128
KC = D // P            # 4 k-chunks of 128
NB = 8                 # 128-row blocks per core
BAND = 4096            # cyclic band (delta 0..31)
HALF = 2048
WIN = 4992             # per-core moving window: 39 blocks
NX = 4                 # delta-32 blocks per core
BIGVAL = 60000.0       # diagonal mask magnitude (exact in fp16)
ENC = 64.0             # norm-encode scale (exact in fp8)

dt = mybir.dt
AF = mybir.ActivationFunctionType


def build_program():
    """Build the per-core Bass/Tile program (identical across cores)."""
    nc = bacc.Bacc("TRN2", num_devices=NCORES, debug=False)

    f8 = dt.float8e4
    # DRAM layout is per-k [KC, P, cols] so each DMA piece moves 128
    # partition-chunks of >=2KB contiguous bytes (fragmented DMAs measure
    # ~20GB/s; contiguous ~250GB/s).  Issues cost ~650ns of engine time
    # each, so they are spread over the sync/gpsimd/tensor/scalar queues
    # (tensor and scalar are otherwise idle during the DMA window).
    xst_d = nc.dram_tensor("xst", [P, KC, NB * P], f8, kind="ExternalInput")
    xmov_d = nc.dram_tensor("xmov", [KC, P, WIN], f8, kind="ExternalInput")
    xx_d = nc.dram_tensor("xx", [P, KC, 2 * NX * P], f8, kind="ExternalInput")
    consts_d = nc.dram_tensor("consts", [P, 16], dt.float32, kind="ExternalInput")
    masks_d = nc.dram_tensor("masks", [P, 2 * P], dt.float16, kind="ExternalInput")
    out_d = nc.dram_tensor("outacc", [P, 27], dt.float32, kind="ExternalOutput")

    with tile.TileContext(nc) as tc:
        with (
            tc.tile_pool(name="big", bufs=1) as bigp,
            tc.tile_pool(name="obuf", bufs=3) as obufp,
            tc.tile_pool(name="small", bufs=1) as smallp,
            tc.tile_pool(name="psum", bufs=2, space="PSUM") as psump,
        ):
            xst = bigp.tile([P, KC, NB * P], f8, tag="xst")
            xmov = bigp.tile([P, KC, WIN], f8, tag="xmov")
            xx = bigp.tile([P, KC, 2 * NX * P], f8, tag="xx")
            consts = smallp.tile([P, 16], dt.float32, tag="consts")
            masks = smallp.tile([P, 2 * P], dt.float16, tag="masks")
            acc = smallp.tile([P, 27], dt.float32, tag="acc")

            # ACT table preload: tiny Derivative_Erf on memset tiles (no DMA
            # deps) so the ~2.7us table load runs during the input DMAs.
            wact = smallp.tile([P, 8], dt.float32, tag="wact")
            wbias = smallp.tile([P, 1], dt.float32, tag="wbias")
            nc.vector.memset(wact[:, :], 1.0)
            nc.vector.memset(wbias[:, :], 0.0)
            nc.scalar.activation(wact[:, :], wact[:, :], AF.Derivative_Erf,
                                 bias=wbias[:, :], scale=1.0)

            # Short PE warmup (HAM clock ramp) while the first DMAs land.
            wident = smallp.tile([P, P], dt.float16, tag="wident")
            wmov = smallp.tile([P, 512], dt.float16, tag="wmov")
            nc.vector.memset(wident[:, :], 1.0)
            nc.vector.memset(wmov[:, :], 1.0)
            warm = psump.tile([P, 512], dt.float32, tag="ps")
            for _ in range(5):
                nc.tensor.matmul(warm[:, :], wident[:, :], wmov[:, :],
                                 start=True, stop=True)

            # Input DMAs: the first-matmul gate (xst[:, :, 0:128] +
            # xmov[:, k, 0:1024] all k) lands first, spread across the
            # three DMA-capable queues; the scalar queue stays nearly free
            # so ACT can start by ~12us.
            nc.sync.dma_start(xst[:, :, 0:P], xst_d[:, :, 0:P])
            nc.gpsimd.dma_start(masks[:], masks_d[:])
            nc.sync.dma_start(xmov[:, 0, 0:1024], xmov_d[0, :, 0:1024])
            nc.gpsimd.dma_start(xmov[:, 1, 0:1024], xmov_d[1, :, 0:1024])
            nc.scalar.dma_start(xmov[:, 3, 0:1024], xmov_d[3, :, 0:1024])
            nc.gpsimd.dma_start(xmov[:, 2, 0:1024], xmov_d[2, :, 0:1024])
            nc.sync.dma_start(consts[:], consts_d[:])
            nc.sync.dma_start(xst[:, :, P:512], xst_d[:, :, P:512])
            nc.sync.dma_start(xmov[:, 0, 1024:3008], xmov_d[0, :, 1024:3008])
            nc.gpsimd.dma_start(xmov[:, 2, 1024:3008], xmov_d[2, :, 1024:3008])
            nc.sync.dma_start(xmov[:, 1, 1024:3008], xmov_d[1, :, 1024:3008])
            nc.gpsimd.dma_start(xmov[:, 3, 1024:3008], xmov_d[3, :, 1024:3008])
            nc.scalar.dma_start(xmov[:, 0, 3008:WIN], xmov_d[0, :, 3008:WIN])
            nc.sync.dma_start(xst[:, :, 512:NB * P], xst_d[:, :, 512:NB * P])
            nc.gpsimd.dma_start(xmov[:, 1, 3008:WIN], xmov_d[1, :, 3008:WIN])
            nc.scalar.dma_start(xmov[:, 2, 3008:WIN], xmov_d[2, :, 3008:WIN])
            nc.gpsimd.dma_start(xmov[:, 3, 3008:WIN], xmov_d[3, :, 3008:WIN])
            nc.sync.dma_start(xx[:], xx_d[:])

            ident = masks[:, 0:P]
            negbig = masks[:, P:2 * P]

            def emit_group(u, g0, glen, acc_col):
                """One psum group: band u, band-cols [g0, g0+glen)."""
                ps = psump.tile([P, glen], dt.float32, tag="ps")
                obuf = obufp.tile([P, glen], dt.float32, tag="ob")
                for lo in range(0, glen, 512):
                    col0 = P * u + g0 + lo
                    nc.tensor.matmul(
                        ps[:, lo:lo + 512],
                        xst[:, 0:2, P * u:P * u + P],
                        xmov[:, 0:2, col0:col0 + 512],
                        start=True, stop=False,
                        perf_mode=mybir.MatmulPerfMode.DoubleRow,
                    )
                    if g0 == 0 and lo == 0:
                        # Mask the true diagonal: psum += I.T@(-BIG*I).
                        nc.tensor.matmul(
                            ps[:, 0:P], ident, negbig,
                            start=False, stop=False,
                        )
                    nc.tensor.matmul(
                        ps[:, lo:lo + 512],
                        xst[:, 2:4, P * u:P * u + P],
                        xmov[:, 2:4, col0:col0 + 512],
                        start=False, stop=True,
                        perf_mode=mybir.MatmulPerfMode.DoubleRow,
                    )
                nc.scalar.activation(
                    obuf[:, :], ps[:, :], AF.Derivative_Erf,
                    bias=consts[:, u:u + 1],
                    scale=consts[:, 8:9],
                    accum_out=acc[:, acc_col:acc_col + 1],
                )
                if g0 == 0:
                    # delta=0 block sums (single-counted correction).
                    nc.vector.tensor_reduce(
                        acc[:, 19 + u:20 + u], obuf[:, 0:P],
                        axis=mybir.AxisListType.X, op=mybir.AluOpType.add,
                    )

            def emit_xgroup():
                """The 4 delta=32 blocks; both norms encoded in-psum."""
                psx = psump.tile([P, NX * P], dt.float32, tag="ps")
                obx = obufp.tile([P, NX * P], dt.float32, tag="ob")
                for k in range(NX):
                    lo = k * P
                    nc.tensor.matmul(
                        psx[:, lo:lo + P],
                        xx[:, 0:2, lo:lo + P], xx[:, 0:2, NX * P + lo:NX * P + lo + P],
                        start=True, stop=False,
                        perf_mode=mybir.MatmulPerfMode.DoubleRow,
                    )
                    nc.tensor.matmul(
                        psx[:, lo:lo + P],
                        xx[:, 2:4, lo:lo + P], xx[:, 2:4, NX * P + lo:NX * P + lo + P],
                        start=False, stop=True,
                        perf_mode=mybir.MatmulPerfMode.DoubleRow,
                    )
                nc.scalar.activation(
                    obx[:, :], psx[:, :], AF.Derivative_Erf,
                    bias=consts[:, 9:10], scale=consts[:, 8:9],
                    accum_out=acc[:, 18:19],
                )

            # Band 0 in four 1024-col groups (ACT pipeline fills early),
            # bands 1-7 in two 2048-col groups; X-group mid-stream (its
            # inputs arrive on the gpsimd queue last).
            for g in range(4):
                emit_group(0, 1024 * g, 1024, g)
            nacc = 4
            for u in range(1, NB):
                for half in range(2):
                    emit_group(u, half * HALF, HALF, nacc)
                    nacc += 1
                if u == 4:
                    emit_xgroup()

            # Ship the [128, 27] accumulator; partition reduction on host.
            nc.sync.dma_start(out_d[:], acc[:])

    nc.finalize()
    return nc


def _fit_parabola(a, x):
    """Weighted LSQ fit of -0.1*sqrt(t) ~ gamma - (alpha*t+beta)^2 over the
    empirical distribution of pairwise squared distances t."""
    rng = np.random.default_rng(12345)
    M = 400_000
    i = rng.integers(0, N, M)
    j = rng.integers(0, N, M)
    keep = i != j
    i, j = i[keep], j[keep]
    xf = x.astype(np.float32)
    t = (a[i] + a[j]
         - 2.0 * np.einsum('ij,ij->i', xf[i], xf[j], optimize=True).astype(np.float64))
    z = 0.1 * np.sqrt(np.maximum(t, 0.0))
    w = np.exp(-z)
    # init: pick gamma0, fit h = sqrt(gamma0+z) affine in t by weighted LSQ
    ga = 6.3
    h0 = np.sqrt(ga + z)
    W = w * w
    A = np.stack([t, np.ones_like(t)], 1)
    AtW = A.T * W
    al, be = np.linalg.solve(AtW @ A, AtW @ h0)

    # Levenberg-Marquardt on r = w*(ga - h^2 + z) (plain GN overshoots).
    def cost(al_, be_, ga_):
        h_ = al_ * t + be_
        r_ = w * (ga_ - h_ * h_ + z)
        return float((r_ * r_).sum())

    lam = 1e-3
    for _ in range(30):
        h = al * t + be
        r = w * (ga - h * h + z)
        J = np.stack([-2 * w * h * t, -2 * w * h, w], 1)
        JTJ = J.T @ J
        g = J.T @ r
        c0 = float((r * r).sum())
        while True:
            Hm = JTJ + lam * np.diag(np.diag(JTJ))
            dlt = np.linalg.solve(Hm, -g)
            cand = (al + dlt[0], be + dlt[1], ga + dlt[2])
            if cost(*cand) <= c0 or lam > 1e12:
                break
            lam *= 10.0
        al, be, ga = cand
        lam = max(lam * 0.3, 1e-12)
    return float(al), float(be), float(ga)


def prepare_inputs(x):
    """Host-side sharding: per-core input dicts for run_bass_kernel_spmd."""
    x = np.ascontiguousarray(np.asarray(x, dtype=np.float32).reshape(N, D))
    a = (x.astype(np.float64) ** 2).sum(axis=1)          # true row norms
    abar = float(a.mean())
    al, be, ga = _fit_parabola(a, x)

    f8 = ml_dtypes.float8_e4m3
    da_enc = ((a - abar) / ENC).astype(f8)               # [N] fp8

    # Moving matrix M [512, N]: rows 0..510 = x dims, row 511 = da_enc.
    MT = np.empty((D, N), dtype=f8)
    MT[0:D - 1] = x.T[0:D - 1].astype(f8)
    MT[D - 1] = da_enc
    # Stationary S [512, N]: rows 0..510 = -2x, row 511 = 64.0.
    ST = np.empty((D, N), dtype=f8)
    ST[0:D - 1] = (-2.0 * x.T[0:D - 1]).astype(f8)
    ST[D - 1] = f8(ENC)
    # X variants: rows 0..509 = dims, plus both norm encodes.
    MXT = np.empty((D, N), dtype=f8)
    MXT[0:D - 2] = MT[0:D - 2]
    MXT[D - 2] = f8(ENC)
    MXT[D - 1] = da_enc
    SXT = np.empty((D, N), dtype=f8)
    SXT[0:D - 2] = ST[0:D - 2]
    SXT[D - 2] = da_enc
    SXT[D - 1] = f8(ENC)

    masks = np.zeros((P, 2 * P), dtype=np.float16)
    masks[:, 0:P] = np.eye(P, dtype=np.float16)
    masks[:, P:2 * P] = (-BIGVAL * np.eye(P)).astype(np.float16)

    in_maps = []
    for c in range(NCORES):
        rows = 1024 * c + np.arange(1024)
        win = (1024 * c + np.arange(WIN)) % N
        rx = 512 * c + np.arange(512)
        cx = rx + 4096
        xst = np.ascontiguousarray(
            ST[:, rows].reshape(KC, P, NB * P).transpose(1, 0, 2))
        xmov = np.ascontiguousarray(MT[:, win].reshape(KC, P, WIN))
        xx = np.empty((P, KC, 2 * NX * P), dtype=f8)
        xx[:, :, 0:NX * P] = SXT[:, rx].reshape(KC, P, NX * P).transpose(1, 0, 2)
        xx[:, :, NX * P:] = MXT[:, cx].reshape(KC, P, NX * P).transpose(1, 0, 2)
        consts = np.zeros((P, 16), dtype=np.float32)
        consts[:, 0:NB] = (al * (a[rows] + abar) + be).astype(np.float32).reshape(NB, P).T
        consts[:, 8] = al
        consts[:, 9] = al * 2.0 * abar + be
        in_maps.append({
            "xst": xst,
            "xmov": xmov,
            "xx": np.ascontiguousarray(xx),
            "consts": consts,
            "masks": masks,
        })
    return in_maps, (al, be, ga)


def combine_outputs(results, ga):
    """Combine per-core [128, 27] accumulators into the final loss values."""
    K = np.exp(ga) * np.sqrt(np.pi) / 2.0
    S = 0.0
    for r in results:
        o = np.asarray(r["outacc"], dtype=np.float64).sum(axis=0)  # [27]
        TX = o[0:19].sum()       # 18 band-group sums + X (all double-counted)
        E0 = o[19:27].sum()      # delta=0 block sums (single-counted)
        S += 2.0 * TX - E0
    total = K * S + float(N)  # exact diagonal (masked to 0 on device)
    loss = 0.1 * total / (float(N) * float(N))
    return np.float32(loss), np.float32(0.5 * loss)


_CACHE = {}


def _get_program():
    if "nc" not in _CACHE:
        _CACHE["nc"] = build_program()
    return _CACHE["nc"]


def run(embeddings, trace=False):
    """Run the Bass kernel on 8 cores; returns (loss, total, BassKernelResults)."""
    nc = _get_program()
    in_maps, (al, be, ga) = prepare_inputs(embeddings)
    res = run_bass_kernel_spmd(nc, in_maps, core_ids=list(range(NCORES)),
                               trace=trace)
    loss, total = combine_outputs(res.results, ga)
    return loss, total, res


def kernel(embeddings):
    loss, total, _ = run(embeddings, trace=False)
    return loss, total


# revision 13
# speedup vs baseline: 1.0817x; 1.0578x over previous
"""Trainium2 Bass kernel for nn_ChallengingGeometricLoss.

Computes loss = 0.1 * mean(exp(-0.1 * cdist(x, x)))  for x = embeddings
reshaped to [N=8192, d=512], plus total = 0.5 * loss.

Key idea: approximate the scalar map  t -> exp(-0.1*sqrt(t))  (t = squared
distance) by  exp(gamma - (alpha*t + beta)^2)  — a least-squares fit of
-0.1*sqrt(t) by a concave parabola over the data's t-range (fit on host
from ~400K sampled pairs; induced relative bias of the mean ~5e-5).  Then
the whole elementwise tail is ONE activation pass:

    g = Derivative_Erf(alpha * psum + bias_i) = (2/sqrt(pi)) * exp(-h^2)

with h = alpha*t + beta assembled by the ACT free affine: psum holds
(a_j - abar) - 2*p_ij from the PE (column norm encoded as an extra fp8
k-row: stationary row 511 = 64.0, moving row 511 = (a_j-abar)/64, x dim
511 dropped — zero-mean noise), and bias_i = alpha*(a_i + abar) + beta is
the per-partition bias.  accum_out gives the band sums for free.

Coverage (8 cores, SPMD): 64 row-blocks of 128; row-block r covers column
blocks (r+delta)%64 for delta in 0..31 (4096 cols; psum double-buffered).
Band 0 is split into 4 [128,1024] psum groups so ACT starts early; bands
1-7 use 2 [128,2048] groups.  The true diagonal of the delta=0 block is
masked with a -60000 PE matmul (h ~ -15 -> g = 0 exactly); delta=0 block
sums (E0) are re-reduced on the idle DVE.  The 32 delta=32 pairs are a
separate uniform X-group (4 [128,128] blocks/core) with BOTH norms encoded
in k-rows 510/511 so the bias is constant.  Host combine:
    sum_full = K*(2*(T+X) - E0) + N,  K = exp(gamma)*sqrt(pi)/2
    loss = 0.1 * sum_full / N^2
"""

import ml_dtypes
import numpy as np

import concourse.bass as bass  # noqa: F401
import concourse.mybir as mybir
import concourse.tile as tile
from concourse import bacc
from concourse.bass_utils import run_bass_kernel_spmd

# Problem constants (hardcoded per contract).
N = 8192
D = 512
NCORES = 8
P = 128
KC = D // P            # 4 k-chunks of 128
NB = 8                 # 128-row blocks per core
BAND = 4096            # cyclic band (delta 0..31)
HALF = 2048
WIN = 4992             # per-core moving window: 39 blocks
NX = 4                 # delta-32 blocks per core
BIGVAL = 60000.0       # diagonal mask magnitude (exact in fp16)
ENC = 64.0             # norm-encode scale (exact in fp8)

dt = mybir.dt
AF = mybir.ActivationFunctionType


def build_program():
    """Build the per-core Bass/Tile program (identical across cores)."""
    nc = bacc.Bacc("TRN2", num_devices=NCORES, debug=False)

    f8 = dt.float8e4
    # DRAM layout is per-k [KC, P, cols] so each DMA piece moves 128
    # partition-chunks of >=2KB contiguous bytes (fragmented DMAs measure
    # ~20GB/s; contiguous ~250GB/s).  Issues cost ~650ns of engine time
    # each, so they are spread over the sync/gpsimd/tensor/scalar queues
    # (tensor and scalar are otherwise idle during the DMA window).
    xst_d = nc.dram_tensor("xst", [P, KC, NB * P], f8, kind="ExternalInput")
    xmov_d = nc.dram_tensor("xmov", [KC, P, WIN], f8, kind="ExternalInput")
    xx_d = nc.dram_tensor("xx", [P, KC, 2 * NX * P], f8, kind="ExternalInput")
    consts_d = nc.dram_tensor("consts", [P, 16], dt.float32, kind="ExternalInput")
    masks_d = nc.dram_tensor("masks", [P, 2 * P], dt.float16, kind="ExternalInput")
    out_d = nc.dram_tensor("outacc", [P, 27], dt.float32, kind="ExternalOutput")

    with tile.TileContext(nc) as tc:
        with (
            tc.tile_pool(name="big", bufs=1) as bigp,
            tc.tile_pool(name="obuf", bufs=3) as obufp,
            tc.tile_pool(name="small", bufs=1) as smallp,
            tc.tile_pool(name="psum", bufs=2, space="PSUM") as psump,
        ):
            xst = bigp.tile([P, KC, NB * P], f8, tag="xst")
            xmov = bigp.tile([P, KC, WIN], f8, tag="xmov")
            xx = bigp.tile([P, KC, 2 * NX * P], f8, tag="xx")
            consts = smallp.tile([P, 16], dt.float32, tag="consts")
            masks = smallp.tile([P, 2 * P], dt.float16, tag="masks")
            acc = smallp.tile([P, 27], dt.float32, tag="acc")

            # ACT table preload: tiny Derivative_Erf on memset tiles (no DMA
            # deps) so the ~2.7us table load runs during the input DMAs.
            wact = smallp.tile([P, 8], dt.float32, tag="wact")
            wbias = smallp.tile([P, 1], dt.float32, tag="wbias")
            nc.vector.memset(wact[:, :], 1.0)
            nc.vector.memset(wbias[:, :], 0.0)
            nc.scalar.activation(wact[:, :], wact[:, :], AF.Derivative_Erf,
                                 bias=wbias[:, :], scale=1.0)

            # Short PE warmup (HAM clock ramp) while the first DMAs land.
            wident = smallp.tile([P, P], dt.float16, tag="wident")
            wmov = smallp.tile([P, 512], dt.float16, tag="wmov")
            nc.vector.memset(wident[:, :], 1.0)
            nc.vector.memset(wmov[:, :], 1.0)
            warm = psump.tile([P, 512], dt.float32, tag="ps")
            for _ in range(5):
                nc.tensor.matmul(warm[:, :], wident[:, :], wmov[:, :],
                                 start=True, stop=True)

            # Input DMAs in need-ordered waves, balanced ~1.15MB per ring
            # (each ring sustains only ~85GB/s; per-k pieces keep 1-2KB
            # contiguous chunks per partition).
            # W0: first-matmul gate.
            nc.sync.dma_start(xst[:, :, 0:P], xst_d[:, :, 0:P])
            nc.gpsimd.dma_start(masks[:], masks_d[:])
            nc.sync.dma_start(xmov[:, 0, 0:1024], xmov_d[0, :, 0:1024])
            nc.gpsimd.dma_start(xmov[:, 1, 0:1024], xmov_d[1, :, 0:1024])
            nc.scalar.dma_start(xmov[:, 3, 0:1024], xmov_d[3, :, 0:1024])
            nc.gpsimd.dma_start(xmov[:, 2, 0:1024], xmov_d[2, :, 0:1024])
            nc.sync.dma_start(consts[:], consts_d[:])
            # W1: band0 tail + band1 head.
            nc.sync.dma_start(xmov[:, 0, 1024:2048], xmov_d[0, :, 1024:2048])
            nc.gpsimd.dma_start(xmov[:, 1, 1024:2048], xmov_d[1, :, 1024:2048])
            nc.scalar.dma_start(xmov[:, 2, 1024:2048], xmov_d[2, :, 1024:2048])
            nc.sync.dma_start(xst[:, :, P:256], xst_d[:, :, P:256])
            nc.scalar.dma_start(xmov[:, 3, 1024:2048], xmov_d[3, :, 1024:2048])
            # W2: middle columns.
            nc.sync.dma_start(xmov[:, 0, 2048:3008], xmov_d[0, :, 2048:3008])
            nc.gpsimd.dma_start(xmov[:, 1, 2048:3008], xmov_d[1, :, 2048:3008])
            nc.gpsimd.dma_start(xmov[:, 2, 2048:3008], xmov_d[2, :, 2048:3008])
            nc.scalar.dma_start(xmov[:, 3, 2048:3008], xmov_d[3, :, 2048:3008])
            # W3: window tail + remaining stationary + X inputs.
            nc.sync.dma_start(xmov[:, 0, 3008:WIN], xmov_d[0, :, 3008:WIN])
            nc.gpsimd.dma_start(xmov[:, 1, 3008:WIN], xmov_d[1, :, 3008:WIN])
            nc.gpsimd.dma_start(xmov[:, 2, 3008:WIN], xmov_d[2, :, 3008:WIN])
            nc.scalar.dma_start(xmov[:, 3, 3008:WIN], xmov_d[3, :, 3008:WIN])
            nc.sync.dma_start(xst[:, :, 256:NB * P], xst_d[:, :, 256:NB * P])
            nc.scalar.dma_start(xx[:], xx_d[:])

            ident = masks[:, 0:P]
            negbig = masks[:, P:2 * P]

            def emit_group(u, g0, glen, acc_col):
                """One psum group: band u, band-cols [g0, g0+glen)."""
                ps = psump.tile([P, glen], dt.float32, tag="ps")
                obuf = obufp.tile([P, glen], dt.float32, tag="ob")
                for lo in range(0, glen, 512):
                    col0 = P * u + g0 + lo
                    nc.tensor.matmul(
                        ps[:, lo:lo + 512],
                        xst[:, 0:2, P * u:P * u + P],
                        xmov[:, 0:2, col0:col0 + 512],
                        start=True, stop=False,
                        perf_mode=mybir.MatmulPerfMode.DoubleRow,
                    )
                    if g0 == 0 and lo == 0:
                        # Mask the true diagonal: psum += I.T@(-BIG*I).
                        nc.tensor.matmul(
                            ps[:, 0:P], ident, negbig,
                            start=False, stop=False,
                        )
                    nc.tensor.matmul(
                        ps[:, lo:lo + 512],
                        xst[:, 2:4, P * u:P * u + P],
                        xmov[:, 2:4, col0:col0 + 512],
                        start=False, stop=True,
                        perf_mode=mybir.MatmulPerfMode.DoubleRow,
                    )
                nc.scalar.activation(
                    obuf[:, :], ps[:, :], AF.Derivative_Erf,
                    bias=consts[:, u:u + 1],
                    scale=consts[:, 8:9],
                    accum_out=acc[:, acc_col:acc_col + 1],
                )
                if g0 == 0:
                    # delta=0 block sums (single-counted correction).
                    nc.vector.tensor_reduce(
                        acc[:, 19 + u:20 + u], obuf[:, 0:P],
                        axis=mybir.AxisListType.X, op=mybir.AluOpType.add,
                    )

            def emit_xgroup():
                """The 4 delta=32 blocks; both norms encoded in-psum."""
                psx = psump.tile([P, NX * P], dt.float32, tag="ps")
                obx = obufp.tile([P, NX * P], dt.float32, tag="ob")
                for k in range(NX):
                    lo = k * P
                    nc.tensor.matmul(
                        psx[:, lo:lo + P],
                        xx[:, 0:2, lo:lo + P], xx[:, 0:2, NX * P + lo:NX * P + lo + P],
                        start=True, stop=False,
                        perf_mode=mybir.MatmulPerfMode.DoubleRow,
                    )
                    nc.tensor.matmul(
                        psx[:, lo:lo + P],
                        xx[:, 2:4, lo:lo + P], xx[:, 2:4, NX * P + lo:NX * P + lo + P],
                        start=False, stop=True,
                        perf_mode=mybir.MatmulPerfMode.DoubleRow,
                    )
                nc.scalar.activation(
                    obx[:, :], psx[:, :], AF.Derivative_Erf,
                    bias=consts[:, 9:10], scale=consts[:, 8:9],
                    accum_out=acc[:, 18:19],
                )

            # Band 0 in four 1024-col groups (ACT pipeline fills early),
            # bands 1-7 in two 2048-col groups; X-group mid-stream (its
            # inputs arrive on the gpsimd queue last).
            for g in range(4):
                emit_group(0, 1024 * g, 1024, g)
            nacc = 4
            for u in range(1, NB):
                for half in range(2):
                    emit_group(u, half * HALF, HALF, nacc)
                    nacc += 1
                if u == 4:
                    emit_xgroup()

            # Ship the [128, 27] accumulator; partition reduction on host.
            nc.sync.dma_start(out_d[:], acc[:])

    nc.finalize()
    return nc


def _fit_parabola(a, x):
    """Weighted LSQ fit of -0.1*sqrt(t) ~ gamma - (alpha*t+beta)^2 over the
    empirical distribution of pairwise squared distances t."""
    rng = np.random.default_rng(12345)
    M = 400_000
    i = rng.integers(0, N, M)
    j = rng.integers(0, N, M)
    keep = i != j
    i, j = i[keep], j[keep]
    xf = x.astype(np.float32)
    t = (a[i] + a[j]
         - 2.0 * np.einsum('ij,ij->i', xf[i], xf[j], optimize=True).astype(np.float64))
    z = 0.1 * np.sqrt(np.maximum(t, 0.0))
    w = np.exp(-z)
    # init: pick gamma0, fit h = sqrt(gamma0+z) affine in t by weighted LSQ
    ga = 6.3
    h0 = np.sqrt(ga + z)
    W = w * w
    A = np.stack([t, np.ones_like(t)], 1)
    AtW = A.T * W
    al, be = np.linalg.solve(AtW @ A, AtW @ h0)

    # Levenberg-Marquardt on r = w*(ga - h^2 + z) (plain GN overshoots).
    def cost(al_, be_, ga_):
        h_ = al_ * t + be_
        r_ = w * (ga_ - h_ * h_ + z)
        return float((r_ * r_).sum())

    lam = 1e-3
    for _ in range(30):
        h = al * t + be
        r = w * (ga - h * h + z)
        J = np.stack([-2 * w * h * t, -2 * w * h, w], 1)
        JTJ = J.T @ J
        g = J.T @ r
        c0 = float((r * r).sum())
        while True:
            Hm = JTJ + lam * np.diag(np.diag(JTJ))
            dlt = np.linalg.solve(Hm, -g)
            cand = (al + dlt[0], be + dlt[1], ga + dlt[2])
            if cost(*cand) <= c0 or lam > 1e12:
                break
            lam *= 10.0
        al, be, ga = cand
        lam = max(lam * 0.3, 1e-12)
    return float(al), float(be), float(ga)


def prepare_inputs(x):
    """Host-side sharding: per-core input dicts for run_bass_kernel_spmd."""
    x = np.ascontiguousarray(np.asarray(x, dtype=np.float32).reshape(N, D))
    a = (x.astype(np.float64) ** 2).sum(axis=1)          # true row norms
    abar = float(a.mean())
    al, be, ga = _fit_parabola(a, x)

    f8 = ml_dtypes.float8_e4m3
    da_enc = ((a - abar) / ENC).astype(f8)               # [N] fp8

    # Moving matrix M [512, N]: rows 0..510 = x dims, row 511 = da_enc.
    MT = np.empty((D, N), dtype=f8)
    MT[0:D - 1] = x.T[0:D - 1].astype(f8)
    MT[D - 1] = da_enc
    # Stationary S [512, N]: rows 0..510 = -2x, row 511 = 64.0.
    ST = np.empty((D, N), dtype=f8)
    ST[0:D - 1] = (-2.0 * x.T[0:D - 1]).astype(f8)
    ST[D - 1] = f8(ENC)
    # X variants: rows 0..509 = dims, plus both norm encodes.
    MXT = np.empty((D, N), dtype=f8)
    MXT[0:D - 2] = MT[0:D - 2]
    MXT[D - 2] = f8(ENC)
    MXT[D - 1] = da_enc
    SXT = np.empty((D, N), dtype=f8)
    SXT[0:D - 2] = ST[0:D - 2]
    SXT[D - 2] = da_enc
    SXT[D - 1] = f8(ENC)

    masks = np.zeros((P, 2 * P), dtype=np.float16)
    masks[:, 0:P] = np.eye(P, dtype=np.float16)
    masks[:, P:2 * P] = (-BIGVAL * np.eye(P)).astype(np.float16)

    in_maps = []
    for c in range(NCORES):
        rows = 1024 * c + np.arange(1024)
        win = (1024 * c + np.arange(WIN)) % N
        rx = 512 * c + np.arange(512)
        cx = rx + 4096
        xst = np.ascontiguousarray(
            ST[:, rows].reshape(KC, P, NB * P).transpose(1, 0, 2))
        xmov = np.ascontiguousarray(MT[:, win].reshape(KC, P, WIN))
        xx = np.empty((P, KC, 2 * NX * P), dtype=f8)
        xx[:, :, 0:NX * P] = SXT[:, rx].reshape(KC, P, NX * P).transpose(1, 0, 2)
        xx[:, :, NX * P:] = MXT[:, cx].reshape(KC, P, NX * P).transpose(1, 0, 2)
        consts = np.zeros((P, 16), dtype=np.float32)
        consts[:, 0:NB] = (al * (a[rows] + abar) + be).astype(np.float32).reshape(NB, P).T
        consts[:, 8] = al
        consts[:, 9] = al * 2.0 * abar + be
        in_maps.append({
            "xst": xst,
            "xmov": xmov,
            "xx": np.ascontiguousarray(xx),
            "consts": consts,
            "masks": masks,
        })
    return in_maps, (al, be, ga)


def combine_outputs(results, ga):
    """Combine per-core [128, 27] accumulators into the final loss values."""
    K = np.exp(ga) * np.sqrt(np.pi) / 2.0
    S = 0.0
    for r in results:
        o = np.asarray(r["outacc"], dtype=np.float64).sum(axis=0)  # [27]
        TX = o[0:19].sum()       # 18 band-group sums + X (all double-counted)
        E0 = o[19:27].sum()      # delta=0 block sums (single-counted)
        S += 2.0 * TX - E0
    total = K * S + float(N)  # exact diagonal (masked to 0 on device)
    loss = 0.1 * total / (float(N) * float(N))
    return np.float32(loss), np.float32(0.5 * loss)


_CACHE = {}


def _get_program():
    if "nc" not in _CACHE:
        _CACHE["nc"] = build_program()
    return _CACHE["nc"]


def run(embeddings, trace=False):
    """Run the Bass kernel on 8 cores; returns (loss, total, BassKernelResults)."""
    nc = _get_program()
    in_maps, (al, be, ga) = prepare_inputs(embeddings)
    res = run_bass_kernel_spmd(nc, in_maps, core_ids=list(range(NCORES)),
                               trace=trace)
    loss, total = combine_outputs(res.results, ga)
    return loss, total, res


def kernel(embeddings):
    loss, total, _ = run(embeddings, trace=False)
    return loss, total
